# revision 21
# baseline (speedup 1.0000x reference)
"""Discretized-mixture NLL loss kernel for Trainium2 (Bass/Tile), 8-core data parallel.

Math (per pixel, per channel c, mixtures m=0..9), matching the reference:
    xhat = W @ px + b            (1x1 conv, 90 outputs = [pi(30) | mu(30) | ls(30)] blocks)
    sigma = exp(8*tanh(ls/8));  s2 = (1/sigma)*(1/sqrt(2)) = exp(-8*tanh(ls/8) + ln(1/sqrt2))
    D = mu - xe                  (xe = image value; folded into the conv via an extra K=4 matmul)
    dcdf = 0.5*(erf((D+d)*s2) - erf((D-d)*s2))          [erf odd]
    num  = sum_m exp(l_m)*dcdf_m ;  den = sum_m exp(l_m)
    nll  = log(den) - log(num + 1e-8*den)

Layout (per core, 16384 pixels = 8 supertiles of 2048). All compute-engine APs
must start at partition 0/32/64/96, so every logical 30-row block is padded to a
32-row block:
  - psum tile [128, 2048] (4 banks): rows 0..31 D, 32..63 ls, 64..95 pi-logits,
    96..127 tanh-out (written by ScalarE). Matmuls are fp32r (full-rate fp32)
    with lhsT [128, 96] over 4 K-chunks plus a K=4 chunk carrying (-xe, bias).
  - ScalarE: tanh; one merged Exp over rows 64..127 (per-partition scale/bias
    AP) producing [g | s2]; Erf over a 2-supertile stacked [128, 2048] tile.
    Activation table sets are phase-batched per pair of supertiles.
  - VectorE: fused scalar_tensor_tensor (D -+ delta)*s2, then q1=g*Elo, q2=g*Ehi.
  - PE mixture-reduction: psum2[8, 2048] = L1.T @ [q1;q2] + L2.T @ g with
    column order [n0 n1 n2 X d0 d1 d2 X]; written into rows 0..7 of the same
    psum tile (D rows are dead by then).
  - ScalarE copy psum2 -> scratch[8, 2048]; one respread DMA -> packed[128, 128]
    column block (partition q = 16r+g). Tail: Ln[128, S*128], one subtract
    (rows 64..127 minus 0..63), DMA rows 0..47 out; host re-interleaves.
"""

import numpy as np

WIDTH = 512
C_IMG = 3
N_MIX = 10
SIZE = 64
STD = 127.5
EPS = 1e-8
DELTA = 1.0 / STD / 2.0
LOG_INV_SQRT2 = -0.34657359027997264
N_CORES = 8
SUP_W = 2048          # pixels per supertile
SUB_W = 512           # matmul moving-dim tile


def make_consts(W, b):
    """Host-side prep of the small constant tensors (32-padded blocks)."""
    W = np.asarray(W, np.float32)
    b = np.asarray(b, np.float32)
    # lhsT column blocks: [mu(30)+2 | ls(30)+2 | pi(30)+2]; intra-block row 3m+c
    Wp = np.zeros((96, WIDTH), np.float32)
    bp = np.zeros(96, np.float32)
    Wp[0:30], bp[0:30] = W[30:60], b[30:60]     # mu
    Wp[32:62], bp[32:62] = W[60:90], b[60:90]   # logsigma
    Wp[64:94], bp[64:94] = W[0:30], b[0:30]     # pi logits
    wt = np.ascontiguousarray(Wp.T)             # [512, 96]
    bx = np.zeros((4, 96), np.float32)          # K=4 chunk rows: (xe0, xe1, xe2, ones)
    for r in range(30):
        bx[r % 3, r] = -1.0                     # D rows get -xe_c
    bx[3, :] = bp                               # ones row carries the conv bias
    # reduction weights; psum2 column order [n0 n1 n2 X d0 d1 d2 X]
    l1 = np.zeros((32, 8), np.float32)          # over qd = g*(Elo-Ehi)
    l2 = np.zeros((32, 8), np.float32)          # over g(32)
    for r in range(30):
        c = r % 3
        l1[r, c] = 0.5                          # +0.5*qd -> num
        l2[r, c] = EPS                          # eps*den folded into num column
        l2[r, 4 + c] = 1.0                      # den
        l2[r, 3] = 1.0                          # dummy cols keep Ln input positive
        l2[r, 7] = 1.0
    scb = np.zeros((64, 2), np.float32)         # merged-exp per-partition (scale, bias)
    scb[0:32, 0] = 1.0                          # pi rows: exp(x)
    scb[32:64, 0] = -8.0                        # tanh rows: exp(-8*t + ln(1/sqrt2))
    scb[32:64, 1] = LOG_INV_SQRT2
    return wt, bx, l1, l2, scb


def build_nc(n_batch=4, use_f32r=True):
    """Build the single-core Bass program (same NEFF runs SPMD on all cores)."""
    from contextlib import ExitStack

    import concourse.bacc as bacc
    import concourse.mybir as mybir
    import concourse.tile as tile
    from concourse.tile import add_dep_helper

    f32 = mybir.dt.float32
    f32r = mybir.dt.float32r
    ALU = mybir.AluOpType
    ACT = mybir.ActivationFunctionType

    def mm_cast(ap):
        return ap.bitcast(f32r) if use_f32r else ap

    S = n_batch * ((SIZE * SIZE) // SUP_W)      # supertiles per core
    assert S % 2 == 0 and S >= 2
    sup_per_batch = (SIZE * SIZE) // SUP_W

    nc = bacc.Bacc("TRN2", target_bir_lowering=False, debug=False)
    pz = nc.dram_tensor("pz", [n_batch, WIDTH, SIZE * SIZE], f32, kind="ExternalInput").ap()
    x4 = nc.dram_tensor("x4", [S, 4, SUP_W], f32, kind="ExternalInput").ap()
    wt = nc.dram_tensor("wt", [WIDTH, 96], f32, kind="ExternalInput").ap()
    bx = nc.dram_tensor("bx", [4, 96], f32, kind="ExternalInput").ap()
    l1 = nc.dram_tensor("l1", [32, 8], f32, kind="ExternalInput").ap()
    l2 = nc.dram_tensor("l2", [32, 8], f32, kind="ExternalInput").ap()
    scb = nc.dram_tensor("scb", [64, 2], f32, kind="ExternalInput").ap()
    out = nc.dram_tensor("out", [48, 128 * S], f32, kind="ExternalOutput").ap()

    with tile.TileContext(nc) as tc, ExitStack() as ctx:
        const_pool = ctx.enter_context(tc.tile_pool(name="const", bufs=1))
        xq_pool = ctx.enter_context(tc.tile_pool(name="xq", bufs=2))
        xt_pool = ctx.enter_context(tc.tile_pool(name="xt", bufs=5))
        s2g_pool = ctx.enter_context(tc.tile_pool(name="s2g", bufs=2))
        hl_pool = ctx.enter_context(tc.tile_pool(name="hl", bufs=2))
        e_pool = ctx.enter_context(tc.tile_pool(name="e", bufs=2))
        qq_pool = ctx.enter_context(tc.tile_pool(name="qq", bufs=2))
        sc_pool = ctx.enter_context(tc.tile_pool(name="sc", bufs=2))
        tail_pool = ctx.enter_context(tc.tile_pool(name="tail", bufs=1))
        ps_pool = ctx.enter_context(tc.tile_pool(name="ps", bufs=2, space="PSUM"))

        # --- constants ---
        wt_sb = const_pool.tile([128, 4 * 96], f32)
        nc.sync.dma_start(
            wt_sb[:].rearrange("i (k o) -> i k o", o=96).bitcast(f32r),
            wt.rearrange("(k i) o -> i k o", i=128).bitcast(f32r),
        )
        bx_sb = const_pool.tile([4, 96], f32)
        nc.sync.dma_start(bx_sb[:].bitcast(f32r), bx.bitcast(f32r))
        l1_sb = const_pool.tile([32, 8], f32)
        nc.sync.dma_start(l1_sb[:].bitcast(f32r), l1.bitcast(f32r))
        l2_sb = const_pool.tile([32, 8], f32)
        nc.sync.dma_start(l2_sb[:].bitcast(f32r), l2.bitcast(f32r))
        scb_sb = const_pool.tile([64, 2], f32)
        nc.sync.dma_start(scb_sb[:], scb)

        packed = tail_pool.tile([128, 128 * S], f32)

        # ACT table-set ordering chain (tanh/exp/erf/ln only; Copy is in every set)
        act_chain = []

        def chain(inst):
            if act_chain:
                add_dep_helper(inst.ins, act_chain[-1].ins, sync=False,
                               reason="act table-set batching")
            act_chain.append(inst)
            return inst

        def phase1(s, hl_t):
            b, h = divmod(s, sup_per_batch)
            half = 64 * (s % 2)
            xq_t = xq_pool.tile([4, SUP_W], f32, tag="xq")
            nc.sync.dma_start(xq_t[:].bitcast(f32r), x4[s].bitcast(f32r))
            xts = []
            for k in range(4):
                xt_t = xt_pool.tile([128, SUP_W], f32, tag="xt")
                nc.sync.dma_start(
                    xt_t[:].bitcast(f32r),
                    pz[b, 128 * k:128 * (k + 1), SUP_W * h:SUP_W * (h + 1)].bitcast(f32r),
                )
                xts.append(xt_t)
            ps = ps_pool.tile([128, SUP_W], f32, tag="ps")
            for t in range(SUP_W // SUB_W):
                sl = slice(SUB_W * t, SUB_W * (t + 1))
                for k in range(4):
                    nc.tensor.matmul(
                        ps[0:96, sl], mm_cast(wt_sb[:, 96 * k:96 * (k + 1)]),
                        mm_cast(xts[k][:, sl]), start=(k == 0), stop=False,
                    )
                nc.tensor.matmul(
                    ps[0:96, sl], mm_cast(bx_sb[:]),
                    mm_cast(xq_t[:, sl]), start=False, stop=True,
                )
            s2g_t = s2g_pool.tile([64, SUP_W], f32, tag="s2g")
            chain(nc.scalar.activation(ps[96:128, :], ps[32:64, :], ACT.Tanh, scale=0.125))
            chain(nc.scalar.activation(
                s2g_t[:].bitcast(f32r), ps[64:128, :], ACT.Exp,
                bias=scb_sb[:, 1:2], scale=scb_sb[:, 0:1],
            ))
            # hi' = (D - delta)*s2 ; lo' = (D + delta)*s2
            hi_t, lo_t = hl_t
            hb = 32 * (s % 2)
            nc.vector.scalar_tensor_tensor(
                hi_t[hb:hb + 32, :], ps[0:32, :], DELTA,
                s2g_t[32:64, :], ALU.subtract, ALU.mult,
            )
            nc.vector.scalar_tensor_tensor(
                lo_t[hb:hb + 32, :], ps[0:32, :], DELTA,
                s2g_t[32:64, :], ALU.add, ALU.mult,
            )
            return ps, s2g_t

        def phase2(s, ps, s2g_t, e_t):
            ehi_t, elo_t = e_t
            hb = 32 * (s % 2)
            g = s2g_t[0:32, :]
            dlt_t = qq_pool.tile([32, SUP_W], f32, tag="dlt")
            nc.vector.tensor_tensor(dlt_t[:], elo_t[hb:hb + 32, :],
                                    ehi_t[hb:hb + 32, :], ALU.subtract)
            qq_t = qq_pool.tile([32, SUP_W], f32, tag="qq")
            nc.vector.tensor_tensor(qq_t[:].bitcast(f32r), g, dlt_t[:], ALU.mult)
            for t in range(SUP_W // SUB_W):
                sl = slice(SUB_W * t, SUB_W * (t + 1))
                nc.tensor.matmul(ps[0:8, sl], mm_cast(l1_sb[:]), mm_cast(qq_t[:, sl]),
                                 start=True, stop=False)
                nc.tensor.matmul(ps[0:8, sl], mm_cast(l2_sb[:]), mm_cast(g[:, sl]),
                                 start=False, stop=True)
            sc_t = sc_pool.tile([8, SUP_W], f32, tag="sc")
            nc.scalar.copy(sc_t[:], ps[0:8, :])
            # respread: (r, g16, p) -> partition q = 16r+g16, column 128s+p
            nc.sync.dma_start(
                packed[:, 128 * s:128 * (s + 1)],
                sc_t[:].rearrange("r (g p) -> r g p", p=128),
            )

        for p in range(S // 2):
            s0, s1 = 2 * p, 2 * p + 1
            hi_t = hl_pool.tile([64, SUP_W], f32, tag="hi")
            lo_t = hl_pool.tile([64, SUP_W], f32, tag="lo")
            ps0, s2g0 = phase1(s0, (hi_t, lo_t))
            ps1, s2g1 = phase1(s1, (hi_t, lo_t))
            ehi_t = e_pool.tile([64, SUP_W], f32, tag="ehi")
            elo_t = e_pool.tile([64, SUP_W], f32, tag="elo")
            chain(nc.scalar.activation(ehi_t[:], hi_t[:], ACT.Erf))
            chain(nc.scalar.activation(elo_t[:], lo_t[:], ACT.Erf))
            phase2(s0, ps0, s2g0, (ehi_t, elo_t))
            phase2(s1, ps1, s2g1, (ehi_t, elo_t))

        # --- tail ---
        ln_n = tail_pool.tile([64, 128 * S], f32)
        ln_d = tail_pool.tile([64, 128 * S], f32)
        chain(nc.scalar.activation(ln_n[:], packed[0:64, :], ACT.Ln))
        chain(nc.scalar.activation(ln_d[:], packed[64:128, :], ACT.Ln))
        nll = tail_pool.tile([64, 128 * S], f32)
        nc.vector.tensor_tensor(nll[:], ln_d[:], ln_n[:], ALU.subtract)
        nc.sync.dma_start(out, nll[0:48, :])

    nc.compile()
    return nc


def prep_core_inputs(px_z_shard, x_shard, consts):
    """px_z_shard [nb, 512, 64, 64], x_shard [nb, 64, 64, 3] -> input map."""
    wt, bx, l1, l2, scb = consts
    nb = px_z_shard.shape[0]
    S = nb * (SIZE * SIZE) // SUP_W
    pzs = np.ascontiguousarray(px_z_shard.reshape(nb, WIDTH, SIZE * SIZE))
    xf = x_shard.reshape(S, SUP_W, C_IMG)
    x4 = np.ones((S, 4, SUP_W), np.float32)
    x4[:, 0:3, :] = xf.transpose(0, 2, 1)
    return {
        "pz": pzs, "x4": np.ascontiguousarray(x4), "wt": wt, "bx": bx,
        "l1": l1, "l2": l2, "scb": scb,
    }


def gather_core_output(o, nb):
    """o [48, 128*S] (row 16c+g, col 128s+p) -> [nb, 64, 64, 3]."""
    S = nb * (SIZE * SIZE) // SUP_W
    return (
        o.reshape(C_IMG, 16, S, 128).transpose(2, 1, 3, 0)
        .reshape(nb, SIZE, SIZE, C_IMG)
    )


_NC_CACHE = {}


def kernel(px_z, x, W, b):
    from concourse.bass_utils import run_bass_kernel_spmd

    px_z = np.asarray(px_z, np.float32)
    x = np.asarray(x, np.float32)
    B = px_z.shape[0]
    nb = B // N_CORES
    consts = make_consts(W, b)
    key = (nb,)
    if key not in _NC_CACHE:
        _NC_CACHE[key] = build_nc(n_batch=nb)
    nc = _NC_CACHE[key]
    in_maps = [
        prep_core_inputs(px_z[nb * i:nb * (i + 1)], x[nb * i:nb * (i + 1)], consts)
        for i in range(N_CORES)
    ]
    res = run_bass_kernel_spmd(nc, in_maps, core_ids=list(range(N_CORES)))
    outs = [gather_core_output(res.results[i]["out"], nb) for i in range(N_CORES)]
    return np.concatenate(outs, 0)


# revision 22
# speedup vs baseline: 1.0822x; 1.0822x over previous
"""Discretized-mixture NLL loss kernel for Trainium2 (Bass/Tile), 8-core data parallel.

Math (per pixel, per channel c, mixtures m=0..9), matching the reference:
    xhat = W @ px + b            (1x1 conv, 90 outputs = [pi(30) | mu(30) | ls(30)] blocks)
    sigma = exp(8*tanh(ls/8));  s2 = (1/sigma)*(1/sqrt(2)) = exp(-8*tanh(ls/8) + ln(1/sqrt2))
    D = mu - xe                  (xe = image value; folded into the conv via an extra K=4 matmul)
    dcdf = 0.5*(erf((D+d)*s2) - erf((D-d)*s2))          [erf odd]
    num  = sum_m exp(l_m)*dcdf_m ;  den = sum_m exp(l_m)
    nll  = log(den) - log(num + 1e-8*den)

Layout (per core, 16384 pixels = 8 supertiles of 2048). All compute-engine APs
must start at partition 0/32/64/96, so every logical 30-row block is padded to a
32-row block:
  - psum tile [128, 2048] (4 banks): rows 0..31 D, 32..63 ls, 64..95 pi-logits,
    96..127 tanh-out (written by ScalarE). Matmuls are fp32r (full-rate fp32)
    with lhsT [128, 96] over 4 K-chunks plus a K=4 chunk carrying (-xe, bias).
  - ScalarE: tanh; one merged Exp over rows 64..127 (per-partition scale/bias
    AP) producing [g | s2]; Erf over a 2-supertile stacked [128, 2048] tile.
    Activation table sets are phase-batched per pair of supertiles.
  - VectorE: fused scalar_tensor_tensor (D -+ delta)*s2, then q1=g*Elo, q2=g*Ehi.
  - PE mixture-reduction: psum2[8, 2048] = L1.T @ [q1;q2] + L2.T @ g with
    column order [n0 n1 n2 X d0 d1 d2 X]; written into rows 0..7 of the same
    psum tile (D rows are dead by then).
  - ScalarE copy psum2 -> scratch[8, 2048]; one respread DMA -> packed[128, 128]
    column block (partition q = 16r+g). Tail: Ln[128, S*128], one subtract
    (rows 64..127 minus 0..63), DMA rows 0..47 out; host re-interleaves.
"""

import numpy as np

WIDTH = 512
C_IMG = 3
N_MIX = 10
SIZE = 64
STD = 127.5
EPS = 1e-8
DELTA = 1.0 / STD / 2.0
LOG_INV_SQRT2 = -0.34657359027997264
N_CORES = 8
SUP_W = 2048          # pixels per supertile
SUB_W = 512           # matmul moving-dim tile


def make_consts(W, b):
    """Host-side prep of the small constant tensors (32-padded blocks)."""
    W = np.asarray(W, np.float32)
    b = np.asarray(b, np.float32)
    # lhsT column blocks: [mu(30)+2 | ls(30)+2 | pi(30)+2]; intra-block row 3m+c
    Wp = np.zeros((96, WIDTH), np.float32)
    bp = np.zeros(96, np.float32)
    Wp[0:30], bp[0:30] = W[30:60], b[30:60]     # mu
    Wp[32:62], bp[32:62] = W[60:90], b[60:90]   # logsigma
    Wp[64:94], bp[64:94] = W[0:30], b[0:30]     # pi logits
    wt = np.ascontiguousarray(Wp.T)             # [512, 96]
    bx = np.zeros((4, 96), np.float32)          # K=4 chunk rows: (xe0, xe1, xe2, ones)
    for r in range(30):
        bx[r % 3, r] = -1.0                     # D rows get -xe_c
    bx[3, :] = bp                               # ones row carries the conv bias
    # reduction weights over s2g = [g(32) | qd(32)];
    # psum2 column order [n0 n1 n2 X d0 d1 d2 X]
    l1 = np.zeros((64, 8), np.float32)
    for r in range(30):
        c = r % 3
        l1[r, c] = EPS                          # eps*den folded into num column
        l1[r, 4 + c] = 1.0                      # den
        l1[r, 3] = 1.0                          # dummy cols keep Ln input positive
        l1[r, 7] = 1.0
        l1[32 + r, c] = 0.5                     # +0.5*qd -> num
    scb = np.zeros((64, 2), np.float32)         # merged-exp per-partition (scale, bias)
    scb[0:32, 0] = 1.0                          # pi rows: exp(x)
    scb[32:64, 0] = -8.0                        # tanh rows: exp(-8*t + ln(1/sqrt2))
    scb[32:64, 1] = LOG_INV_SQRT2
    return wt, bx, l1, scb


def build_nc(n_batch=4, use_f32r=True):
    """Build the single-core Bass program (same NEFF runs SPMD on all cores)."""
    from contextlib import ExitStack

    import concourse.bacc as bacc
    import concourse.mybir as mybir
    import concourse.tile as tile
    from concourse.tile import add_dep_helper

    f32 = mybir.dt.float32
    f32r = mybir.dt.float32r
    ALU = mybir.AluOpType
    ACT = mybir.ActivationFunctionType

    def mm_cast(ap):
        return ap.bitcast(f32r) if use_f32r else ap

    S = n_batch * ((SIZE * SIZE) // SUP_W)      # supertiles per core
    assert S % 2 == 0 and S >= 2
    sup_per_batch = (SIZE * SIZE) // SUP_W

    nc = bacc.Bacc("TRN2", target_bir_lowering=False, debug=False)
    pz = nc.dram_tensor("pz", [n_batch, WIDTH, SIZE * SIZE], f32, kind="ExternalInput").ap()
    x4 = nc.dram_tensor("x4", [S, 4, SUP_W], f32, kind="ExternalInput").ap()
    wt = nc.dram_tensor("wt", [WIDTH, 96], f32, kind="ExternalInput").ap()
    bx = nc.dram_tensor("bx", [4, 96], f32, kind="ExternalInput").ap()
    l1 = nc.dram_tensor("l1", [64, 8], f32, kind="ExternalInput").ap()
    scb = nc.dram_tensor("scb", [64, 2], f32, kind="ExternalInput").ap()
    out = nc.dram_tensor("out", [48, 128 * S], f32, kind="ExternalOutput").ap()

    with tile.TileContext(nc) as tc, ExitStack() as ctx:
        const_pool = ctx.enter_context(tc.tile_pool(name="const", bufs=1))
        xq_pool = ctx.enter_context(tc.tile_pool(name="xq", bufs=2))
        xt_pool = ctx.enter_context(tc.tile_pool(name="xt", bufs=5))
        s2g_pool = ctx.enter_context(tc.tile_pool(name="s2g", bufs=2))
        hl_pool = ctx.enter_context(tc.tile_pool(name="hl", bufs=2))
        e_pool = ctx.enter_context(tc.tile_pool(name="e", bufs=2))
        qq_pool = ctx.enter_context(tc.tile_pool(name="qq", bufs=2))
        sc_pool = ctx.enter_context(tc.tile_pool(name="sc", bufs=2))
        tail_pool = ctx.enter_context(tc.tile_pool(name="tail", bufs=1))
        ps_pool = ctx.enter_context(tc.tile_pool(name="ps", bufs=2, space="PSUM"))

        # --- constants ---
        wt_sb = const_pool.tile([128, 4 * 96], f32)
        nc.sync.dma_start(
            wt_sb[:].rearrange("i (k o) -> i k o", o=96).bitcast(f32r),
            wt.rearrange("(k i) o -> i k o", i=128).bitcast(f32r),
        )
        bx_sb = const_pool.tile([4, 96], f32)
        nc.sync.dma_start(bx_sb[:].bitcast(f32r), bx.bitcast(f32r))
        l1_sb = const_pool.tile([64, 8], f32)
        nc.sync.dma_start(l1_sb[:].bitcast(f32r), l1.bitcast(f32r))
        scb_sb = const_pool.tile([64, 2], f32)
        nc.sync.dma_start(scb_sb[:], scb)

        packed = tail_pool.tile([128, 128 * S], f32)

        # ACT table-set ordering chain (tanh/exp/erf/ln only; Copy is in every set)
        act_chain = []

        def chain(inst):
            if act_chain:
                add_dep_helper(inst.ins, act_chain[-1].ins, sync=False,
                               reason="act table-set batching")
            act_chain.append(inst)
            return inst

        def phase1(s, hl_t):
            b, h = divmod(s, sup_per_batch)
            half = 64 * (s % 2)
            xq_t = xq_pool.tile([4, SUP_W], f32, tag="xq")
            nc.sync.dma_start(xq_t[:].bitcast(f32r), x4[s].bitcast(f32r))
            xts = []
            for k in range(4):
                xt_t = xt_pool.tile([128, SUP_W], f32, tag="xt")
                nc.sync.dma_start(
                    xt_t[:].bitcast(f32r),
                    pz[b, 128 * k:128 * (k + 1), SUP_W * h:SUP_W * (h + 1)].bitcast(f32r),
                )
                xts.append(xt_t)
            ps = ps_pool.tile([128, SUP_W], f32, tag="ps")
            for t in range(SUP_W // SUB_W):
                sl = slice(SUB_W * t, SUB_W * (t + 1))
                for k in range(4):
                    nc.tensor.matmul(
                        ps[0:96, sl], mm_cast(wt_sb[:, 96 * k:96 * (k + 1)]),
                        mm_cast(xts[k][:, sl]), start=(k == 0), stop=False,
                    )
                nc.tensor.matmul(
                    ps[0:96, sl], mm_cast(bx_sb[:]),
                    mm_cast(xq_t[:, sl]), start=False, stop=True,
                )
            s2g_t = s2g_pool.tile([64, SUP_W], f32, tag="s2g")
            chain(nc.scalar.activation(ps[96:128, :], ps[32:64, :], ACT.Tanh, scale=0.125))
            chain(nc.scalar.activation(
                s2g_t[:].bitcast(f32r), ps[64:128, :], ACT.Exp,
                bias=scb_sb[:, 1:2], scale=scb_sb[:, 0:1],
            ))
            # hi' = (D - delta)*s2 ; lo' = (D + delta)*s2
            hi_t, lo_t = hl_t
            hb = 32 * (s % 2)
            nc.vector.scalar_tensor_tensor(
                hi_t[hb:hb + 32, :], ps[0:32, :], DELTA,
                s2g_t[32:64, :], ALU.subtract, ALU.mult,
            )
            nc.vector.scalar_tensor_tensor(
                lo_t[hb:hb + 32, :], ps[0:32, :], DELTA,
                s2g_t[32:64, :], ALU.add, ALU.mult,
            )
            return ps, s2g_t

        def phase2(s, ps, s2g_t, e_t):
            ehi_t, elo_t = e_t
            hb = 32 * (s % 2)
            g = s2g_t[0:32, :]
            dlt_t = qq_pool.tile([32, SUP_W], f32, tag="dlt")
            nc.vector.tensor_tensor(dlt_t[:], elo_t[hb:hb + 32, :],
                                    ehi_t[hb:hb + 32, :], ALU.subtract)
            # qd = g*dlt overwrites the dead s2 rows -> [g | qd] contiguous
            nc.vector.tensor_tensor(s2g_t[32:64, :].bitcast(f32r), g, dlt_t[:], ALU.mult)
            for t in range(SUP_W // SUB_W):
                sl = slice(SUB_W * t, SUB_W * (t + 1))
                nc.tensor.matmul(ps[0:8, sl], mm_cast(l1_sb[:]), mm_cast(s2g_t[:, sl]),
                                 start=True, stop=True)
            sc_t = sc_pool.tile([8, SUP_W], f32, tag="sc")
            nc.scalar.copy(sc_t[:], ps[0:8, :])
            # respread: (r, g16, p) -> partition q = 16r+g16, column 128s+p
            nc.sync.dma_start(
                packed[:, 128 * s:128 * (s + 1)],
                sc_t[:].rearrange("r (g p) -> r g p", p=128),
            )

        for p in range(S // 2):
            s0, s1 = 2 * p, 2 * p + 1
            hi_t = hl_pool.tile([64, SUP_W], f32, tag="hi")
            lo_t = hl_pool.tile([64, SUP_W], f32, tag="lo")
            ps0, s2g0 = phase1(s0, (hi_t, lo_t))
            ps1, s2g1 = phase1(s1, (hi_t, lo_t))
            ehi_t = e_pool.tile([64, SUP_W], f32, tag="ehi")
            elo_t = e_pool.tile([64, SUP_W], f32, tag="elo")
            chain(nc.scalar.activation(ehi_t[:], hi_t[:], ACT.Erf))
            chain(nc.scalar.activation(elo_t[:], lo_t[:], ACT.Erf))
            phase2(s0, ps0, s2g0, (ehi_t, elo_t))
            phase2(s1, ps1, s2g1, (ehi_t, elo_t))

        # --- tail ---
        ln_n = tail_pool.tile([64, 128 * S], f32)
        ln_d = tail_pool.tile([64, 128 * S], f32)
        chain(nc.scalar.activation(ln_n[:], packed[0:64, :], ACT.Ln))
        chain(nc.scalar.activation(ln_d[:], packed[64:128, :], ACT.Ln))
        nll = tail_pool.tile([64, 128 * S], f32)
        nc.vector.tensor_tensor(nll[:], ln_d[:], ln_n[:], ALU.subtract)
        nc.sync.dma_start(out, nll[0:48, :])

    nc.compile()
    return nc


def prep_core_inputs(px_z_shard, x_shard, consts):
    """px_z_shard [nb, 512, 64, 64], x_shard [nb, 64, 64, 3] -> input map."""
    wt, bx, l1, scb = consts
    nb = px_z_shard.shape[0]
    S = nb * (SIZE * SIZE) // SUP_W
    pzs = np.ascontiguousarray(px_z_shard.reshape(nb, WIDTH, SIZE * SIZE))
    xf = x_shard.reshape(S, SUP_W, C_IMG)
    x4 = np.ones((S, 4, SUP_W), np.float32)
    x4[:, 0:3, :] = xf.transpose(0, 2, 1)
    return {
        "pz": pzs, "x4": np.ascontiguousarray(x4), "wt": wt, "bx": bx,
        "l1": l1, "scb": scb,
    }


def gather_core_output(o, nb):
    """o [48, 128*S] (row 16c+g, col 128s+p) -> [nb, 64, 64, 3]."""
    S = nb * (SIZE * SIZE) // SUP_W
    return (
        o.reshape(C_IMG, 16, S, 128).transpose(2, 1, 3, 0)
        .reshape(nb, SIZE, SIZE, C_IMG)
    )


_NC_CACHE = {}


def kernel(px_z, x, W, b):
    from concourse.bass_utils import run_bass_kernel_spmd

    px_z = np.asarray(px_z, np.float32)
    x = np.asarray(x, np.float32)
    B = px_z.shape[0]
    nb = B // N_CORES
    consts = make_consts(W, b)
    key = (nb,)
    if key not in _NC_CACHE:
        _NC_CACHE[key] = build_nc(n_batch=nb)
    nc = _NC_CACHE[key]
    in_maps = [
        prep_core_inputs(px_z[nb * i:nb * (i + 1)], x[nb * i:nb * (i + 1)], consts)
        for i in range(N_CORES)
    ]
    res = run_bass_kernel_spmd(nc, in_maps, core_ids=list(range(N_CORES)))
    outs = [gather_core_output(res.results[i]["out"], nb) for i in range(N_CORES)]
    return np.concatenate(outs, 0)
